# revision 1
# baseline (speedup 1.0000x reference)
"""BLSTM Trainium2 kernel: 8-core SPMD.

Sharding: core pair q={2q,2q+1} owns batch element q (41 frames of width 200).
Even core runs the forward 2-layer LSTM chain, odd core the backward chain
(host feeds it time-reversed frames). One pairwise AllGather exchanges the
final-layer hidden streams; each core then projects (Wp), overlap-adds its
half of the time axis, adds the skip connection, and returns a padded
[512, 2300] slice that the host trims and concatenates.
"""
import numpy as np
from contextlib import ExitStack

U = 512
S = 200          # frame width (LSTM steps)
F = 41           # frames per batch element
T = 4200
STRIDE = 100
COLS = S * F     # 8200 time-major columns per core
G = 4 * U        # 2048 gate rows
NCORES = 8
SEG = 2100       # half of T, per-core output segment
PAD = 100
ACC_W = SEG + 2 * PAD  # 2300
KT = U // 128    # 4 k-tiles
MT = G // 128    # 16 m-tiles
FR_LOC = 21      # frames projected per core (1 overlap frame)

_CACHE = {}


def _build():
    import os
    steps = int(os.environ.get("BL_STEPS", S))
    do_coll = os.environ.get("BL_COLLECTIVE", "1") == "1"
    do_dyn = os.environ.get("BL_DYN", "1") == "1"
    do_proj = os.environ.get("BL_PROJ", "1") == "1"
    import concourse.bacc as bacc
    import concourse.tile as tile
    import concourse.bass as bass
    from concourse import mybir

    f32 = mybir.dt.float32
    f16 = mybir.dt.float16
    AF = mybir.ActivationFunctionType

    nc = bacc.Bacc("TRN2", target_bir_lowering=False, debug=False,
                   num_devices=NCORES)

    xT = nc.dram_tensor("xT", [U, COLS], f16, kind="ExternalInput")
    Wx0 = nc.dram_tensor("Wx0", [U, G], f16, kind="ExternalInput")
    Wh0 = nc.dram_tensor("Wh0", [U, G], f16, kind="ExternalInput")
    Wx1 = nc.dram_tensor("Wx1", [U, G], f16, kind="ExternalInput")
    Wh1 = nc.dram_tensor("Wh1", [U, G], f16, kind="ExternalInput")
    b0d = nc.dram_tensor("b0", [G, 1], f32, kind="ExternalInput")
    b1d = nc.dram_tensor("b1", [G, 1], f32, kind="ExternalInput")
    Wpd = nc.dram_tensor("Wp", [2 * U, U], f16, kind="ExternalInput")
    bpd = nc.dram_tensor("bp", [U, 1], f32, kind="ExternalInput")
    skipd = nc.dram_tensor("skip", [U, ACC_W], f32, kind="ExternalInput")
    eyed = nc.dram_tensor("eye", [128, 128], f16, kind="ExternalInput")
    outd = nc.dram_tensor("out", [U, ACC_W], f32, kind="ExternalOutput")

    with ExitStack() as ctx:
        tc = ctx.enter_context(tile.TileContext(nc))
        # persistent pools
        wpool = ctx.enter_context(tc.tile_pool(name="w", bufs=1))
        big = ctx.enter_context(tc.tile_pool(name="big", bufs=1))
        state = ctx.enter_context(tc.tile_pool(name="state", bufs=3))
        dram = ctx.enter_context(tc.tile_pool(name="dram", bufs=1, space="DRAM"))

        pid = nc.partition_id()
        parity = pid % 2

        # ---- dram scratch (zx split into step-aligned chunks: 492 = 12*41)
        ZCH = 492
        NZC = 17
        zx0_d = [dram.tile([G, min(ZCH, COLS - c * ZCH)], f16, tag=f"zx0_{c}",
                           name=f"zx0_{c}")
                 for c in range(NZC)]
        zx1_d = [dram.tile([G, min(ZCH, COLS - c * ZCH)], f16, tag=f"zx1_{c}",
                           name=f"zx1_{c}")
                 for c in range(NZC)]
        h1_d = dram.tile([U, COLS], f16)
        gth_d = dram.tile([2 * U, COLS], f16)

        # ---- load + cast weights to fp16
        def load_w16(src, kt, cols, tag):
            tiles = []
            for k in range(kt):
                t16 = wpool.tile([128, cols], f16, tag=f"{tag}{k}",
                                 name=f"w_{tag}{k}")
                nc.sync.dma_start(t16[:], src[k * 128:(k + 1) * 128, :])
                tiles.append(t16)
            return tiles

        wx0 = load_w16(Wx0, KT, G, "wx0")
        wh0 = load_w16(Wh0, KT, G, "wh0")
        wx1 = load_w16(Wx1, KT, G, "wx1")
        wh1 = load_w16(Wh1, KT, G, "wh1")
        wp = load_w16(Wpd, 8, U, "wp")

        eye = wpool.tile([128, 128], f16, tag="eye")
        nc.sync.dma_start(eye[:], eyed[:])

        b0t = wpool.tile([128, MT], f32, tag="b0")
        b1t = wpool.tile([128, MT], f32, tag="b1")
        for m in range(MT):
            nc.sync.dma_start(b0t[:, m:m + 1], b0d[m * 128:(m + 1) * 128, :])
            nc.sync.dma_start(b1t[:, m:m + 1], b1d[m * 128:(m + 1) * 128, :])
        bpt = wpool.tile([128, 4], f32, tag="bp")
        for m in range(4):
            nc.sync.dma_start(bpt[:, m:m + 1], bpd[m * 128:(m + 1) * 128, :])

        # ---- load x (already fp16), k-major [128, KT*COLS]
        xh = big.tile([128, KT * COLS], f16, tag="big")
        for k in range(KT):
            nc.sync.dma_start(xh[:, k * COLS:(k + 1) * COLS],
                              xT[k * 128:(k + 1) * 128, :])

        # ---- zx GEMM: dst[g, c] = sum_u W[u, g] * rhs[u, c] + b[g]
        def zx_gemm(dst_tiles, wtiles, rhs_tile, rhs_stride, btile):
            ZCH = 492
            with tc.tile_pool(name="zxg", bufs=4) as zp, \
                 tc.tile_pool(name="zxp", bufs=6, space="PSUM") as pp:
                for c, dst in enumerate(dst_tiles):
                    c0 = c * ZCH
                    cn = dst.shape[1]
                    for m in range(MT):
                        ps = pp.tile([128, ZCH], f32, tag="ps")
                        for k in range(KT):
                            nc.tensor.matmul(
                                ps[:, :cn],
                                wtiles[k][:, m * 128:(m + 1) * 128],
                                rhs_tile[:, k * rhs_stride + c0:
                                         k * rhs_stride + c0 + cn],
                                start=(k == 0), stop=(k == KT - 1))
                        zt = zp.tile([128, ZCH], f16, tag="zt")
                        nc.scalar.activation(zt[:, :cn], ps[:, :cn],
                                             AF.Identity,
                                             bias=btile[:, m:m + 1])
                        nc.sync.dma_start(dst[m * 128:(m + 1) * 128, :cn],
                                          zt[:, :cn])

        zx_gemm(zx0_d, wx0, xh, COLS, b0t)

        # ---- LSTM recurrence
        # h layout: [128, KT*41] k-major fp16; c: [128, KT*41] f32
        def recurrence(zx_tiles, whtiles, h_sink):
            # h_sink(step, h_ap_src) emits the store of the new h
            zx_rs = [z[:].rearrange("(m p) c -> p m c", p=128)
                     for z in zx_tiles]
            with tc.tile_pool(name="rec", bufs=6) as rp, \
                 tc.tile_pool(name="recg", bufs=4) as gp, \
                 tc.tile_pool(name="rech", bufs=4) as hp, \
                 tc.tile_pool(name="recp", bufs=8, space="PSUM") as pp:
                czero = state.tile([128, KT * F], f32, tag="c")
                nc.vector.memset(czero[:], 0.0)
                hprev = hp.tile([128, KT * F], f16, tag="h")
                nc.vector.memset(hprev[:], 0.0)
                cprev = czero
                for s in range(steps):
                    zxs = rp.tile([128, MT * F], f16, tag="zxs")
                    so = (s % 12) * F
                    nc.sync.dma_start(zxs[:],
                                      zx_rs[s // 12][:, :, so:so + F])
                    gates = {}
                    for gi, gname in enumerate(("i", "f", "g", "o")):
                        ps = pp.tile([128, 4 * F], f32, tag="ps")
                        nc.tensor.matmul(ps[:], eye[:],
                                         zxs[:, gi * 4 * F:(gi + 1) * 4 * F],
                                         start=True, stop=False)
                        for jj in range(4):
                            mcol = gi * 512 + jj * 128
                            for k in range(KT):
                                nc.tensor.matmul(
                                    ps[:, jj * F:(jj + 1) * F],
                                    whtiles[k][:, mcol:mcol + 128],
                                    hprev[:, k * F:(k + 1) * F],
                                    start=False,
                                    stop=(k == KT - 1 and jj == 3),
                                    skip_group_check=True)
                        gt = gp.tile([128, 4 * F], f32, tag=f"g{gname}")
                        nc.scalar.activation(
                            gt[:], ps[:],
                            AF.Tanh if gname == "g" else AF.Sigmoid)
                        gates[gname] = gt
                    t1 = gp.tile([128, 4 * F], f32, tag="t1")
                    nc.vector.tensor_mul(t1[:], gates["i"][:], gates["g"][:])
                    cnew = state.tile([128, KT * F], f32, tag="c")
                    nc.vector.tensor_mul(cnew[:], gates["f"][:], cprev[:])
                    nc.vector.tensor_add(cnew[:], cnew[:], t1[:])
                    tcr = gp.tile([128, 4 * F], f32, tag="tc")
                    nc.scalar.activation(tcr[:], cnew[:], AF.Tanh)
                    hnew = hp.tile([128, KT * F], f16, tag="h")
                    nc.vector.tensor_mul(hnew[:], gates["o"][:], tcr[:])
                    h_sink(s, hnew)
                    hprev = hnew
                    cprev = cnew

        # f0: h goes straight into SBUF (feeds zx1 gemm)
        h0 = big.tile([128, KT * COLS], f16, tag="big")

        def sink0(s, hnew):
            nc.vector.tensor_copy(
                h0[:].rearrange("p (k c) -> p k c", k=KT)[:, :, s * F:(s + 1) * F],
                hnew[:].rearrange("p (k c) -> p k c", k=KT))

        recurrence(zx0_d, wh0, sink0)

        zx_gemm(zx1_d, wx1, h0, COLS, b1t)

        # f1: h streams to DRAM, frame-major with per-core time (un)reversal:
        # col = f*S + (parity ? S-1-s : s)
        h1_r = h1_d[:].rearrange("(k p) (f s) -> p k f s", p=128, s=S)
        sbase = parity * (S - 1)
        smul = 1 - 2 * parity

        def sink1(s, hnew):
            if do_dyn:
                off = nc.s_assert_within(sbase + s * smul, 0, S - 1,
                                         skip_runtime_assert=True)
                for k in range(KT):
                    nc.sync.dma_start(
                        h1_r[:, k, :, bass.ds(off, 1)],
                        hnew[:, k * F:(k + 1) * F])
            else:
                for k in range(KT):
                    nc.sync.dma_start(
                        h1_r[:, k, :, s:s + 1],
                        hnew[:, k * F:(k + 1) * F])

        recurrence(zx1_d, wh1, sink1)

        # ---- exchange h1 within the pair
        if do_coll:
            nc.gpsimd.collective_compute(
                "AllGather", mybir.AluOpType.bypass,
                replica_groups=[[0, 1], [2, 3], [4, 5], [6, 7]],
                ins=[h1_d[:]], outs=[gth_d[:]])
        else:
            nc.sync.dma_start(gth_d[0:U, :], h1_d[:])
            nc.sync.dma_start(gth_d[U:2 * U, :], h1_d[:])

        # ---- projection + overlap-add
        accum = big.tile([128, 4 * ACC_W], f32, tag="big")
        for k in range(4):
            nc.sync.dma_start(accum[:, k * ACC_W:(k + 1) * ACC_W],
                              skipd[k * 128:(k + 1) * 128, :])

        gth_r = gth_d[:].rearrange("(kk p) c -> p kk c", p=128)
        f0off = parity * 20 * S  # element offset into the column dim
        with tc.tile_pool(name="prj", bufs=3) as jp, \
             tc.tile_pool(name="prp", bufs=4, space="PSUM") as pp:
            for j in range(FR_LOC if do_proj else 0):
                rhs = jp.tile([128, 8 * S], f16, tag="rhs")
                nc.sync.dma_start(
                    rhs[:].rearrange("p (kk s) -> p kk s", kk=8),
                    gth_r[:, :, bass.ds(f0off + j * S, S)])
                for m in range(4):
                    ps = pp.tile([128, S], f32, tag="ps")
                    for k in range(8):
                        nc.tensor.matmul(
                            ps[:], wp[k][:, m * 128:(m + 1) * 128],
                            rhs[:, k * S:(k + 1) * S],
                            start=(k == 0), stop=(k == 7))
                    pt = jp.tile([128, S], f32, tag="pt")
                    nc.scalar.activation(pt[:], ps[:], AF.Identity,
                                         bias=bpt[:, m:m + 1])
                    a0 = j * STRIDE
                    nc.vector.tensor_add(
                        accum[:, m * ACC_W + a0:m * ACC_W + a0 + S],
                        accum[:, m * ACC_W + a0:m * ACC_W + a0 + S],
                        pt[:])

        # ---- output
        for k in range(4):
            nc.sync.dma_start(outd[k * 128:(k + 1) * 128, :],
                              accum[:, k * ACC_W:(k + 1) * ACC_W])

    nc.compile()
    return nc


def _prep_inputs(inputs, Wx_f0, Wh_f0, b_f0, Wx_f1, Wh_f1, b_f1,
                 Wx_b0, Wh_b0, b_b0, Wx_b1, Wh_b1, b_b1, Wp, bp):
    x = np.asarray(inputs, dtype=np.float32)  # [4, 512, 4200]
    eye = np.eye(128, dtype=np.float16)
    idx = np.arange(F)[:, None] * STRIDE + np.arange(S)[None, :]  # [F, S]
    wsets = {
        0: (Wx_f0, Wh_f0, b_f0, Wx_f1, Wh_f1, b_f1),
        1: (Wx_b0, Wh_b0, b_b0, Wx_b1, Wh_b1, b_b1),
    }
    in_maps = []
    for c in range(NCORES):
        q, parity = c // 2, c % 2
        xs = x[q][:, idx]                       # [U, F, S]
        if parity:
            xs = xs[:, :, ::-1]
        xTc = np.ascontiguousarray(
            xs.transpose(0, 2, 1).reshape(U, COLS)).astype(np.float16)
        wx0, wh0, b0, wx1, wh1, b1 = wsets[parity]
        # skip goes only into the kept window (pads are trimmed by the host)
        sk = np.zeros((U, ACC_W), dtype=np.float32)
        if parity == 0:
            sk[:, 0:SEG] = x[q][:, 0:SEG]          # kept window [0:2100)
        else:
            sk[:, PAD:PAD + SEG] = x[q][:, SEG:T]  # kept window [100:2200)
        in_maps.append({
            "xT": xTc,
            "Wx0": np.asarray(wx0, np.float16),
            "Wh0": np.asarray(wh0, np.float16),
            "Wx1": np.asarray(wx1, np.float16),
            "Wh1": np.asarray(wh1, np.float16),
            "b0": np.asarray(b0, np.float32).reshape(G, 1),
            "b1": np.asarray(b1, np.float32).reshape(G, 1),
            "Wp": np.asarray(Wp, np.float16),
            "bp": np.asarray(bp, np.float32).reshape(U, 1),
            "skip": sk,
            "eye": eye,
        })
    return in_maps


def kernel(**inputs) -> np.ndarray:
    from concourse.bass_utils import run_bass_kernel_spmd

    if "nc" not in _CACHE:
        _CACHE["nc"] = _build()
    nc = _CACHE["nc"]

    import os
    in_maps = _prep_inputs(**inputs)
    trace = os.environ.get("BL_TRACE", "0") == "1"
    res = run_bass_kernel_spmd(nc, in_maps, list(range(NCORES)), trace=trace)
    _CACHE["last_result"] = res

    out = np.zeros((4, U, T), dtype=np.float32)
    for c in range(NCORES):
        q, parity = c // 2, c % 2
        seg = res.results[c]["out"]  # [U, ACC_W]
        if parity == 0:
            out[q][:, 0:SEG] = seg[:, 0:SEG]
        else:
            out[q][:, SEG:T] = seg[:, PAD:PAD + SEG]
    return out



# revision 2
# speedup vs baseline: 2.9708x; 2.9708x over previous
"""BLSTM Trainium2 kernel: 8-core SPMD, SBUF-resident wavefront.

Core pair q={2q,2q+1} owns batch element q; even core runs the forward
2-layer LSTM chain, odd the backward chain (host feeds time-reversed
frames). Both layers are software-pipelined (L1 lags L0 by 2 steps) on
one core; Wx@x_t and the bias are fused into each step's gate PSUM
accumulation, so nothing round-trips through DRAM. The final-layer
hidden stream stays in SBUF (a dynamic-offset copy un-reverses time on
odd cores), is projected per-frame through this core's half of Wp, and
overlap-added into an fp16 accumulator seeded with skip+bias. The pair
exchanges projected partials; both cores emit the identical full
[512, 4200] output (host reads the even core's).
"""
import numpy as np
from contextlib import ExitStack

U = 512
S = 200          # frame width (LSTM steps)
F = 41           # frames per batch element
T = 4200
STRIDE = 100
COLS = S * F     # 8200 time-major columns per core (col = s*41 + f)
G = 4 * U        # 2048 gate rows
NCORES = 8
KT = U // 128    # 4 k-tiles
KF = KT * F      # 164 cols per step (k-major hidden layout)
CH = 12          # steps per x chunk
NCH = (S + CH - 1) // CH   # 17 chunks
LAG = 2          # L1 wavefront lag

_CACHE = {}


def _build():
    import os
    steps = int(os.environ.get("BL_STEPS", S))
    do_coll = os.environ.get("BL_COLLECTIVE", "1") == "1"
    import concourse.bacc as bacc
    import concourse.tile as tile
    import concourse.bass as bass
    from concourse import mybir

    f32 = mybir.dt.float32
    f16 = mybir.dt.float16
    AF = mybir.ActivationFunctionType

    nc = bacc.Bacc("TRN2", target_bir_lowering=False, debug=False,
                   num_devices=NCORES)

    xT = nc.dram_tensor("xT", [U, COLS], f16, kind="ExternalInput")
    Wx0 = nc.dram_tensor("Wx0", [U, G], f16, kind="ExternalInput")
    Wh0 = nc.dram_tensor("Wh0", [U, G], f16, kind="ExternalInput")
    Wx1 = nc.dram_tensor("Wx1", [U, G], f16, kind="ExternalInput")
    Wh1 = nc.dram_tensor("Wh1", [U, G], f16, kind="ExternalInput")
    bb0d = nc.dram_tensor("bb0", [128, 2 * 328], f16, kind="ExternalInput")
    bb1d = nc.dram_tensor("bb1", [128, 2 * 328], f16, kind="ExternalInput")
    Wpd = nc.dram_tensor("Wp", [U, U], f16, kind="ExternalInput")
    skipd = nc.dram_tensor("skip", [U, T], f16, kind="ExternalInput")
    eyed = nc.dram_tensor("eye", [128, 128], f16, kind="ExternalInput")
    outd = nc.dram_tensor("out", [U, T], f16, kind="ExternalOutput")

    with ExitStack() as ctx:
        tc = ctx.enter_context(tile.TileContext(nc))
        wpool = ctx.enter_context(tc.tile_pool(name="w", bufs=1))
        h1pool = ctx.enter_context(tc.tile_pool(name="h1sb", bufs=1))
        dram = ctx.enter_context(tc.tile_pool(name="dram", bufs=1, space="DRAM"))

        pid = nc.partition_id()
        parity = pid % 2
        sbase = parity * (S - 1)
        smul = 1 - 2 * parity

        partial_d = dram.tile([U, T], f16)
        gth_d = dram.tile([2 * U, T], f16)

        # ---- persistent weights
        def load_w16(src, kt, cols, tag):
            tiles = []
            for k in range(kt):
                t16 = wpool.tile([128, cols], f16, tag=f"{tag}{k}",
                                 name=f"w_{tag}{k}")
                nc.sync.dma_start(t16[:], src[k * 128:(k + 1) * 128, :])
                tiles.append(t16)
            return tiles

        wx0 = load_w16(Wx0, KT, G, "wx0")
        wh0 = load_w16(Wh0, KT, G, "wh0")
        wx1 = load_w16(Wx1, KT, G, "wx1")
        wh1 = load_w16(Wh1, KT, G, "wh1")
        wp = load_w16(Wpd, KT, U, "wp")

        eye = wpool.tile([128, 128], f16, tag="eye")
        nc.sync.dma_start(eye[:], eyed[:])
        bb0 = wpool.tile([128, 656], f16, tag="bb0")
        nc.sync.dma_start(bb0[:], bb0d[:])
        bb1 = wpool.tile([128, 656], f16, tag="bb1")
        nc.sync.dma_start(bb1[:], bb1d[:])

        # final-layer hidden stream, [128, (s kf)] fp16
        h1_sb = h1pool.tile([128, S * KF], f16, tag="h1sb")
        h1v = h1_sb[:].rearrange("p (s kf) -> p s kf", kf=KF)

        GATE_OF_TILE = ((0, 1), (2, 3))  # psum tile A=(i,f), B=(g,o)

        with tc.tile_pool(name="xp", bufs=3) as xp, \
             tc.tile_pool(name="hr", bufs=6) as hp, \
             tc.tile_pool(name="cr", bufs=3) as cp, \
             tc.tile_pool(name="gp", bufs=4) as gp, \
             tc.tile_pool(name="zz", bufs=1) as zz, \
             tc.tile_pool(name="pp", bufs=8, space="PSUM") as pp:

            hz = zz.tile([128, KF], f16, tag="hz")
            nc.vector.memset(hz[:], 0.0)
            cz0 = zz.tile([128, KF], f32, tag="cz0")
            nc.vector.memset(cz0[:], 0.0)
            cz1 = zz.tile([128, KF], f32, tag="cz1")
            nc.vector.memset(cz1[:], 0.0)

            xchunks = [None] * NCH

            def load_chunk(c):
                n = min(CH, steps - c * CH) * F
                xc = xp.tile([128, KT * CH * F], f16, tag="xc")
                for k in range(KT):
                    nc.sync.dma_start(
                        xc[:, k * CH * F:k * CH * F + n],
                        xT[k * 128:(k + 1) * 128,
                           c * CH * F:c * CH * F + n])
                xchunks[c] = xc

            def emit_step(layer, s, hprev, cprev, xrhs, wx, wh, bb):
                psA = pp.tile([128, 328], f32, tag="ps")
                psB = pp.tile([128, 328], f32, tag="ps")
                ps = (psA, psB)
                nc.tensor.matmul(psA[:], eye[:], bb[:, 0:328],
                                 start=True, stop=False)
                nc.tensor.matmul(psB[:], eye[:], bb[:, 328:656],
                                 start=True, stop=False)
                # input injection (no recurrent dep)
                for ti in range(2):
                    for gi, g in enumerate(GATE_OF_TILE[ti]):
                        for m in range(4):
                            col = g * 512 + m * 128
                            dst = ps[ti][:, gi * 164 + m * F:
                                         gi * 164 + (m + 1) * F]
                            for k in range(KT):
                                nc.tensor.matmul(
                                    dst, wx[k][:, col:col + 128], xrhs(k),
                                    start=False, stop=False,
                                    skip_group_check=True)
                # recurrent part; stop on each tile's last matmul
                for ti in range(2):
                    for gi, g in enumerate(GATE_OF_TILE[ti]):
                        for m in range(4):
                            col = g * 512 + m * 128
                            dst = ps[ti][:, gi * 164 + m * F:
                                         gi * 164 + (m + 1) * F]
                            for k in range(KT):
                                nc.tensor.matmul(
                                    dst, wh[k][:, col:col + 128],
                                    hprev[:, k * F:(k + 1) * F],
                                    start=False,
                                    stop=(gi == 1 and m == 3 and k == KT - 1),
                                    skip_group_check=True)
                sig_if = gp.tile([128, 328], f16, tag=f"if{layer}")
                nc.scalar.activation(sig_if[:], psA[:], AF.Sigmoid)
                g16 = gp.tile([128, KF], f16, tag=f"g{layer}")
                nc.scalar.activation(g16[:], psB[:, 0:164], AF.Tanh)
                o16 = gp.tile([128, KF], f16, tag=f"o{layer}")
                nc.scalar.activation(o16[:], psB[:, 164:328], AF.Sigmoid)
                t1 = gp.tile([128, KF], f32, tag=f"t1{layer}")
                nc.vector.tensor_mul(t1[:], sig_if[:, 0:164], g16[:])
                cnew = cp.tile([128, KF], f32, tag=f"c{layer}")
                nc.vector.tensor_mul(cnew[:], sig_if[:, 164:328], cprev[:])
                nc.vector.tensor_add(cnew[:], cnew[:], t1[:])
                tc16 = gp.tile([128, KF], f16, tag=f"tc{layer}")
                nc.scalar.activation(tc16[:], cnew[:], AF.Tanh)
                hnew = hp.tile([128, KF], f16, tag=f"h{layer}")
                nc.vector.tensor_mul(hnew[:], o16[:], tc16[:])
                return hnew, cnew

            h0s, c0s = hz, cz0
            h1s, c1s = hz, cz1
            h0bystep = {}
            for w in range(steps + LAG):
                if w < steps:
                    if w % CH == 0:
                        load_chunk(w // CH)
                    xc = xchunks[w // CH]
                    so = (w % CH) * F
                    h0s, c0s = emit_step(
                        0, w, h0s, c0s,
                        lambda k: xc[:, k * CH * F + so:k * CH * F + so + F],
                        wx0, wh0, bb0)
                    h0bystep[w] = h0s
                if w >= LAG:
                    s = w - LAG
                    h0in = h0bystep.pop(s)
                    h1s, c1s = emit_step(
                        1, s, h1s, c1s,
                        lambda k: h0in[:, k * F:(k + 1) * F],
                        wx1, wh1, bb1)
                    off = nc.s_assert_within(sbase + s * smul, 0, S - 1,
                                             skip_runtime_assert=True)
                    nc.vector.tensor_copy(
                        h1v[:, bass.ds(off, 1), :],
                        h1s[:].rearrange("p (one kf) -> p one kf", one=1))

        # ---- projection + overlap-add (accum seeded with skip + bp)
        accum = h1pool.tile([128, 4 * T], f16, tag="acc")
        for m in range(4):
            nc.sync.dma_start(accum[:, m * T:(m + 1) * T],
                              skipd[m * 128:(m + 1) * 128, :])

        h1f = h1_sb[:].rearrange("p (s kf) -> p kf s", kf=KF)
        with tc.tile_pool(name="prp", bufs=8, space="PSUM") as ppp:
            for f in range(F):
                for m in range(4):
                    psP = ppp.tile([128, S], f32, tag="pp")
                    for k in range(KT):
                        nc.tensor.matmul(
                            psP[:], wp[k][:, m * 128:(m + 1) * 128],
                            h1f[:, k * F + f, :],
                            start=(k == 0), stop=(k == KT - 1))
                    a0 = m * T + f * STRIDE
                    nc.vector.tensor_add(accum[:, a0:a0 + S],
                                         accum[:, a0:a0 + S], psP[:])

        # ---- pair exchange of projected partials
        for m in range(4):
            nc.sync.dma_start(partial_d[m * 128:(m + 1) * 128, :],
                              accum[:, m * T:(m + 1) * T])
        if do_coll:
            nc.gpsimd.collective_compute(
                "AllGather", mybir.AluOpType.bypass,
                replica_groups=[[0, 1], [2, 3], [4, 5], [6, 7]],
                ins=[partial_d[:]], outs=[gth_d[:]])
        else:
            nc.sync.dma_start(gth_d[0:U, :], partial_d[:])
            nc.sync.dma_start(gth_d[U:2 * U, :], partial_d[:])

        # add the peer's partial into accum in place, then store
        peer = 1 - parity
        gth_v = gth_d[:].rearrange("(two u) t -> two u t", two=2)
        with tc.tile_pool(name="fin", bufs=2) as fp:
            for m in range(4):
                b = fp.tile([128, T], f16, tag="b")
                nc.sync.dma_start(
                    b[:], gth_v[bass.ds(peer, 1), m * 128:(m + 1) * 128, :])
                a0 = m * T
                nc.vector.tensor_add(accum[:, a0:a0 + T],
                                     accum[:, a0:a0 + T], b[:])
                nc.sync.dma_start(outd[m * 128:(m + 1) * 128, :],
                                  accum[:, a0:a0 + T])

    nc.compile()
    return nc


def _prep_inputs(inputs, Wx_f0, Wh_f0, b_f0, Wx_f1, Wh_f1, b_f1,
                 Wx_b0, Wh_b0, b_b0, Wx_b1, Wh_b1, b_b1, Wp, bp):
    x = np.asarray(inputs, dtype=np.float32)  # [4, 512, 4200]
    eye = np.eye(128, dtype=np.float16)
    idx = np.arange(F)[:, None] * STRIDE + np.arange(S)[None, :]  # [F, S]
    wsets = {
        0: (Wx_f0, Wh_f0, b_f0, Wx_f1, Wh_f1, b_f1),
        1: (Wx_b0, Wh_b0, b_b0, Wx_b1, Wh_b1, b_b1),
    }

    def bias_bcast(b):
        # [128, 656]: cols [ti*328 + gi*164 + m*41 + j] = b[g*512 + m*128 + p]
        b = np.asarray(b, np.float32).reshape(4, 4, 128)  # [gate, m, p]
        out = np.zeros((128, 656), np.float16)
        for ti, gates in enumerate(((0, 1), (2, 3))):
            for gi, g in enumerate(gates):
                for m in range(4):
                    c0 = ti * 328 + gi * 164 + m * F
                    out[:, c0:c0 + F] = b[g, m][:, None]
        return out

    Wp = np.asarray(Wp, np.float32)           # [2U, U]
    bp = np.asarray(bp, np.float32)
    in_maps = []
    for c in range(NCORES):
        q, parity = c // 2, c % 2
        xs = x[q][:, idx]                       # [U, F, S]
        if parity:
            xs = xs[:, :, ::-1]
        xTc = np.ascontiguousarray(
            xs.transpose(0, 2, 1).reshape(U, COLS)).astype(np.float16)
        wx0, wh0, b0, wx1, wh1, b1 = wsets[parity]
        if parity == 0:
            sk = (x[q] + bp[:, None]).astype(np.float16)
        else:
            sk = np.zeros((U, T), dtype=np.float16)
        in_maps.append({
            "xT": xTc,
            "Wx0": np.asarray(wx0, np.float16),
            "Wh0": np.asarray(wh0, np.float16),
            "Wx1": np.asarray(wx1, np.float16),
            "Wh1": np.asarray(wh1, np.float16),
            "bb0": bias_bcast(b0),
            "bb1": bias_bcast(b1),
            "Wp": Wp[parity * U:(parity + 1) * U, :].astype(np.float16),
            "skip": sk,
            "eye": eye,
        })
    return in_maps


def kernel(**inputs) -> np.ndarray:
    from concourse.bass_utils import run_bass_kernel_spmd

    if "nc" not in _CACHE:
        _CACHE["nc"] = _build()
    nc = _CACHE["nc"]

    import os
    in_maps = _prep_inputs(**inputs)
    trace = os.environ.get("BL_TRACE", "0") == "1"
    res = run_bass_kernel_spmd(nc, in_maps, list(range(NCORES)), trace=trace)
    _CACHE["last_result"] = res

    out = np.zeros((4, U, T), dtype=np.float32)
    for q in range(4):
        out[q] = res.results[2 * q]["out"]
    return out


# revision 5
# speedup vs baseline: 3.3081x; 1.1136x over previous
"""BLSTM Trainium2 kernel: 8-core SPMD, SBUF-resident wavefront.

Core pair q={2q,2q+1} owns batch element q; even core runs the forward
2-layer LSTM chain, odd the backward chain (host feeds time-reversed
frames). Both layers are software-pipelined (L1 lags L0 by 2 steps) on
one core; Wx@x_t and the bias are fused into each step's gate PSUM
accumulation, so nothing round-trips through DRAM. The final-layer
hidden stream stays in SBUF (a dynamic-offset copy un-reverses time on
odd cores), is projected per-frame through this core's half of Wp, and
overlap-added into an fp16 accumulator seeded with skip+bias. The pair
exchanges projected partials; both cores emit the identical full
[512, 4200] output (host reads the even core's).
"""
import numpy as np
from contextlib import ExitStack

U = 512
S = 200          # frame width (LSTM steps)
F = 41           # frames per batch element
T = 4200
STRIDE = 100
COLS = S * F     # 8200 time-major columns per core (col = s*41 + f)
G = 4 * U        # 2048 gate rows
NCORES = 8
KT = U // 128    # 4 k-tiles
KF = KT * F      # 164 cols per step (k-major hidden layout)
CH = 12          # steps per x chunk
NCH = (S + CH - 1) // CH   # 17 chunks
LAG = 2          # L1 wavefront lag

_CACHE = {}


def _build():
    import os
    steps = int(os.environ.get("BL_STEPS", S))
    do_coll = os.environ.get("BL_COLLECTIVE", "1") == "1"
    dr_mode = os.environ.get("BL_DR", "x")
    dr_x = dr_mode in ("1", "x")
    dr_h = dr_mode in ("1", "h")
    dr_wx1 = dr_h or dr_mode == "wx1"   # L1 input projection (h0 rhs)
    dr_wh = dr_h or dr_mode == "wh"     # recurrent Wh (h rhs)
    do_dr = dr_x or dr_wx1 or dr_wh
    import concourse.bacc as bacc
    import concourse.tile as tile
    import concourse.bass as bass
    from concourse import mybir

    f32 = mybir.dt.float32
    f16 = mybir.dt.float16
    f8 = mybir.dt.float8e4
    AF = mybir.ActivationFunctionType
    xdt = f8 if dr_x else f16
    hdt = f8 if (dr_wx1 or dr_wh) else f16   # h tiles feed both consumers

    nc = bacc.Bacc("TRN2", target_bir_lowering=False, debug=False,
                   num_devices=NCORES)

    xT = nc.dram_tensor("xT", [U, COLS], xdt, kind="ExternalInput")
    Wx0 = nc.dram_tensor("Wx0", [U, G], xdt, kind="ExternalInput")
    whdt = f8 if dr_wh else f16
    wx1dt = f8 if dr_wx1 else f16
    Wh0 = nc.dram_tensor("Wh0", [U, G], whdt, kind="ExternalInput")
    Wx1 = nc.dram_tensor("Wx1", [U, G], wx1dt, kind="ExternalInput")
    Wh1 = nc.dram_tensor("Wh1", [U, G], whdt, kind="ExternalInput")
    bb0d = nc.dram_tensor("bb0", [128, 2 * 328], f16, kind="ExternalInput")
    bb1d = nc.dram_tensor("bb1", [128, 2 * 328], f16, kind="ExternalInput")
    Wpd = nc.dram_tensor("Wp", [U, U], f16, kind="ExternalInput")
    skipd = nc.dram_tensor("skip", [U, T], f32, kind="ExternalInput")
    eyed = nc.dram_tensor("eye", [128, 128], f16, kind="ExternalInput")
    outd = nc.dram_tensor("out", [U, T], f32, kind="ExternalOutput")

    with ExitStack() as ctx:
        tc = ctx.enter_context(tile.TileContext(nc))
        wpool = ctx.enter_context(tc.tile_pool(name="w", bufs=1))
        h1pool = ctx.enter_context(tc.tile_pool(name="h1sb", bufs=1))
        dram = ctx.enter_context(tc.tile_pool(name="dram", bufs=1, space="DRAM"))

        pid = nc.partition_id()
        parity = pid % 2
        sbase = parity * (S - 1)
        smul = 1 - 2 * parity

        partial_d = dram.tile([U, T], f16)
        gth_d = dram.tile([2 * U, T], f16)

        # ---- persistent weights (only what the tail phase needs)
        def load_w16(pool, src, kt, cols, tag):
            tiles = []
            for k in range(kt):
                t16 = pool.tile([128, cols], f16, tag=f"{tag}{k}",
                                name=f"w_{tag}{k}")
                nc.sync.dma_start(t16[:], src[k * 128:(k + 1) * 128, :])
                tiles.append(t16)
            return tiles

        def load_wpair(pool, src, kt, cols, tag):
            # adjacent k-tile pairs [128, 2*cols] for DoubleRow lhsT planes
            tiles = []
            for t2 in range(kt // 2):
                t16 = pool.tile([128, 2 * cols], f8, tag=f"{tag}p{t2}",
                                name=f"w_{tag}p{t2}")
                for j in range(2):
                    k = 2 * t2 + j
                    nc.sync.dma_start(t16[:, j * cols:(j + 1) * cols],
                                      src[k * 128:(k + 1) * 128, :])
                tiles.append(t16)
            return tiles

        DR = mybir.MatmulPerfMode.DoubleRow

        wp = load_w16(wpool, Wpd, KT, U, "wp")
        eye = wpool.tile([128, 128], f16, tag="eye")
        nc.sync.dma_start(eye[:], eyed[:])

        # final-layer hidden stream, [128, (s kf)] fp16
        h1_sb = h1pool.tile([128, S * KF], f16, tag="h1sb")
        h1v = h1_sb[:].rearrange("p (s kf) -> p s kf", kf=KF)

        GATE_OF_TILE = ((0, 1), (2, 3))  # psum tile A=(i,f), B=(g,o)

        with tc.tile_pool(name="wrec", bufs=1) as wrec, \
             tc.tile_pool(name="xp", bufs=3) as xp, \
             tc.tile_pool(name="hr", bufs=6) as hp, \
             tc.tile_pool(name="cr", bufs=3) as cp, \
             tc.tile_pool(name="gp", bufs=2) as gp, \
             tc.tile_pool(name="zz", bufs=1) as zz, \
             tc.tile_pool(name="pp", bufs=8, space="PSUM") as pp:

            load_x = load_wpair if dr_x else load_w16
            wx0 = load_x(wrec, Wx0, KT, G, "wx0")
            wh0 = (load_wpair if dr_wh else load_w16)(wrec, Wh0, KT, G, "wh0")
            wx1 = (load_wpair if dr_wx1 else load_w16)(wrec, Wx1, KT, G, "wx1")
            wh1 = (load_wpair if dr_wh else load_w16)(wrec, Wh1, KT, G, "wh1")
            bb0 = wrec.tile([128, 656], f16, tag="bb0")
            nc.sync.dma_start(bb0[:], bb0d[:])
            bb1 = wrec.tile([128, 656], f16, tag="bb1")
            nc.sync.dma_start(bb1[:], bb1d[:])

            hz = zz.tile([128, KF], hdt, tag="hz")
            nc.vector.memset(hz[:], 0.0)
            cz0 = zz.tile([128, KF], f32, tag="cz0")
            nc.vector.memset(cz0[:], 0.0)
            cz1 = zz.tile([128, KF], f32, tag="cz1")
            nc.vector.memset(cz1[:], 0.0)

            xchunks = [None] * NCH

            def load_chunk(c):
                n = min(CH, steps - c * CH) * F
                xc = xp.tile([128, KT * CH * F], xdt, tag="xc")
                for k in range(KT):
                    nc.sync.dma_start(
                        xc[:, k * CH * F:k * CH * F + n],
                        xT[k * 128:(k + 1) * 128,
                           c * CH * F:c * CH * F + n])
                xchunks[c] = xc

            def emit_step(layer, s, hprev, cprev, xrhs, wx, wh, bb):
                psA = pp.tile([128, 328], f32, tag="ps")
                psB = pp.tile([128, 328], f32, tag="ps")
                ps = (psA, psB)
                nc.tensor.matmul(psA[:], eye[:], bb[:, 0:328],
                                 start=True, stop=False)
                nc.tensor.matmul(psB[:], eye[:], bb[:, 328:656],
                                 start=True, stop=False)
                def wslc(wt, t2, col):
                    return wt[t2][:].rearrange(
                        "p (j g) -> p j g", j=2)[:, :, col:col + 128]

                def gemm(wtiles, rhs2, rhs1, use_dr, is_last):
                    for ti in range(2):
                        for gi, g in enumerate(GATE_OF_TILE[ti]):
                            for m in range(4):
                                col = g * 512 + m * 128
                                dst = ps[ti][:, gi * 164 + m * F:
                                             gi * 164 + (m + 1) * F]
                                if use_dr:
                                    for t2 in range(KT // 2):
                                        nc.tensor.matmul(
                                            dst, wslc(wtiles, t2, col),
                                            rhs2(t2), start=False,
                                            stop=(is_last and gi == 1
                                                  and m == 3 and t2 == 1),
                                            perf_mode=DR,
                                            skip_group_check=True)
                                else:
                                    for k in range(KT):
                                        nc.tensor.matmul(
                                            dst,
                                            wtiles[k][:, col:col + 128],
                                            rhs1(k), start=False,
                                            stop=(is_last and gi == 1
                                                  and m == 3 and k == KT - 1),
                                            skip_group_check=True)

                # input injection (no recurrent dep), then recurrent part
                gemm(wx, xrhs[0], xrhs[1], xrhs[2], False)
                if dr_wh:
                    hrhs = hprev[:].rearrange(
                        "p (t2 j f) -> p t2 j f", t2=2, j=2)
                    gemm(wh, lambda t2: hrhs[:, t2], None, True, True)
                else:
                    gemm(wh, None,
                         lambda k: hprev[:, k * F:(k + 1) * F], False, True)
                gsc = 1.0 / 16.0 if do_dr else 1.0  # uniform weight scaling
                sig_if = gp.tile([128, 328], f32, tag=f"if{layer}")
                nc.scalar.activation(sig_if[:], psA[:], AF.Sigmoid, scale=gsc)
                g32 = gp.tile([128, KF], f32, tag=f"g{layer}")
                nc.scalar.activation(g32[:], psB[:, 0:164], AF.Tanh, scale=gsc)
                o32 = gp.tile([128, KF], f32, tag=f"o{layer}")
                nc.scalar.activation(o32[:], psB[:, 164:328], AF.Sigmoid,
                                     scale=gsc)
                t1 = gp.tile([128, KF], f32, tag=f"t1{layer}")
                nc.vector.tensor_mul(t1[:], sig_if[:, 0:164], g32[:])
                cnew = cp.tile([128, KF], f32, tag=f"c{layer}")
                nc.vector.tensor_mul(cnew[:], sig_if[:, 164:328], cprev[:])
                nc.vector.tensor_add(cnew[:], cnew[:], t1[:])
                tc32 = gp.tile([128, KF], f32, tag=f"tc{layer}")
                nc.scalar.activation(tc32[:], cnew[:], AF.Tanh)
                hnew = hp.tile([128, KF], hdt, tag=f"h{layer}")
                nc.vector.tensor_mul(hnew[:], o32[:], tc32[:])
                return hnew, cnew

            h0s, c0s = hz, cz0
            h1s, c1s = hz, cz1
            h0bystep = {}
            for w in range(steps + LAG):
                if w < steps:
                    if w % CH == 0:
                        load_chunk(w // CH)
                    xc = xchunks[w // CH]
                    so = (w % CH) * F
                    xcr = xc[:].rearrange(
                        "p (t2 j sf) -> p t2 j sf", t2=2, j=2)
                    xrhs = ((lambda t2, xcr=xcr, so=so:
                             xcr[:, t2, :, so:so + F]),
                            (lambda k, xc=xc, so=so:
                             xc[:, k * CH * F + so:k * CH * F + so + F]),
                            dr_x)
                    h0s, c0s = emit_step(0, w, h0s, c0s, xrhs,
                                         wx0, wh0, bb0)
                    h0bystep[w] = h0s
                if w >= LAG:
                    s = w - LAG
                    h0in = h0bystep.pop(s)
                    h0r = h0in[:].rearrange(
                        "p (t2 j f) -> p t2 j f", t2=2, j=2)
                    xrhs1 = ((lambda t2, h0r=h0r: h0r[:, t2]),
                             (lambda k, h0in=h0in:
                              h0in[:, k * F:(k + 1) * F]),
                             dr_wx1)
                    h1s, c1s = emit_step(1, s, h1s, c1s, xrhs1,
                                         wx1, wh1, bb1)
                    off = nc.s_assert_within(sbase + s * smul, 0, S - 1,
                                             skip_runtime_assert=True)
                    nc.vector.tensor_copy(
                        h1v[:, bass.ds(off, 1), :],
                        h1s[:].rearrange("p (one kf) -> p one kf", one=1))

        # ---- projection + overlap-add (accum seeded with skip + bp)
        tailp = ctx.enter_context(tc.tile_pool(name="tail", bufs=1))
        accum = tailp.tile([128, 4 * T], f32, tag="acc")
        for m in range(4):
            nc.sync.dma_start(accum[:, m * T:(m + 1) * T],
                              skipd[m * 128:(m + 1) * 128, :])

        h1f = h1_sb[:].rearrange("p (s kf) -> p kf s", kf=KF)
        with tc.tile_pool(name="prp", bufs=8, space="PSUM") as ppp:
            for f in range(F):
                for m in range(4):
                    psP = ppp.tile([128, S], f32, tag="pp")
                    for k in range(KT):
                        nc.tensor.matmul(
                            psP[:], wp[k][:, m * 128:(m + 1) * 128],
                            h1f[:, k * F + f, :],
                            start=(k == 0), stop=(k == KT - 1))
                    a0 = m * T + f * STRIDE
                    nc.vector.tensor_add(accum[:, a0:a0 + S],
                                         accum[:, a0:a0 + S], psP[:])

        # ---- pair exchange of projected partials (converted to fp16)
        with tc.tile_pool(name="cvt", bufs=2) as cvt:
            for m in range(4):
                p16 = cvt.tile([128, T], f16, tag="p16")
                nc.vector.tensor_copy(p16[:], accum[:, m * T:(m + 1) * T])
                nc.sync.dma_start(partial_d[m * 128:(m + 1) * 128, :], p16[:])
        if do_coll:
            nc.gpsimd.collective_compute(
                "AllGather", mybir.AluOpType.bypass,
                replica_groups=[[0, 1], [2, 3], [4, 5], [6, 7]],
                ins=[partial_d[:]], outs=[gth_d[:]])
        else:
            nc.sync.dma_start(gth_d[0:U, :], partial_d[:])
            nc.sync.dma_start(gth_d[U:2 * U, :], partial_d[:])

        # add the peer's partial into accum in place, then store
        peer = 1 - parity
        gth_v = gth_d[:].rearrange("(two u) t -> two u t", two=2)
        with tc.tile_pool(name="fin", bufs=2) as fp:
            for m in range(4):
                b = fp.tile([128, T], f16, tag="b")
                nc.sync.dma_start(
                    b[:], gth_v[bass.ds(peer, 1), m * 128:(m + 1) * 128, :])
                a0 = m * T
                nc.vector.tensor_add(accum[:, a0:a0 + T],
                                     accum[:, a0:a0 + T], b[:])
                nc.sync.dma_start(outd[m * 128:(m + 1) * 128, :],
                                  accum[:, a0:a0 + T])

    nc.compile()
    return nc


def _prep_inputs(inputs, Wx_f0, Wh_f0, b_f0, Wx_f1, Wh_f1, b_f1,
                 Wx_b0, Wh_b0, b_b0, Wx_b1, Wh_b1, b_b1, Wp, bp):
    import os
    dr_mode = os.environ.get("BL_DR", "x")
    import ml_dtypes
    f8np = ml_dtypes.float8_e4m3
    xnp = f8np if dr_mode in ("1", "x") else np.float16
    wx1np = f8np if dr_mode in ("1", "h", "wx1") else np.float16
    whnp = f8np if dr_mode in ("1", "h", "wh") else np.float16
    wscale = 16.0 if dr_mode != "0" else 1.0
    x = np.asarray(inputs, dtype=np.float32)  # [4, 512, 4200]
    eye = np.eye(128, dtype=np.float16)
    idx = np.arange(F)[:, None] * STRIDE + np.arange(S)[None, :]  # [F, S]
    wsets = {
        0: (Wx_f0, Wh_f0, b_f0, Wx_f1, Wh_f1, b_f1),
        1: (Wx_b0, Wh_b0, b_b0, Wx_b1, Wh_b1, b_b1),
    }

    def bias_bcast(b):
        # [128, 656]: cols [ti*328 + gi*164 + m*41 + j] = b[g*512 + m*128 + p]
        b = (np.asarray(b, np.float32) * wscale).reshape(4, 4, 128)
        out = np.zeros((128, 656), np.float16)
        for ti, gates in enumerate(((0, 1), (2, 3))):
            for gi, g in enumerate(gates):
                for m in range(4):
                    c0 = ti * 328 + gi * 164 + m * F
                    out[:, c0:c0 + F] = b[g, m][:, None]
        return out

    Wp = np.asarray(Wp, np.float32)           # [2U, U]
    bp = np.asarray(bp, np.float32)
    in_maps = []
    for c in range(NCORES):
        q, parity = c // 2, c % 2
        xs = x[q][:, idx]                       # [U, F, S]
        if parity:
            xs = xs[:, :, ::-1]
        xTc = np.ascontiguousarray(
            xs.transpose(0, 2, 1).reshape(U, COLS)).astype(xnp)
        wx0, wh0, b0, wx1, wh1, b1 = wsets[parity]
        if parity == 0:
            sk = (x[q] + bp[:, None]).astype(np.float32)
        else:
            sk = np.zeros((U, T), dtype=np.float32)
        in_maps.append({
            "xT": xTc,
            "Wx0": (np.asarray(wx0, np.float32) * wscale).astype(xnp),
            "Wh0": (np.asarray(wh0, np.float32) * wscale).astype(whnp),
            "Wx1": (np.asarray(wx1, np.float32) * wscale).astype(wx1np),
            "Wh1": (np.asarray(wh1, np.float32) * wscale).astype(whnp),
            "bb0": bias_bcast(b0),
            "bb1": bias_bcast(b1),
            "Wp": Wp[parity * U:(parity + 1) * U, :].astype(np.float16),
            "skip": sk,
            "eye": eye,
        })
    return in_maps


def kernel(**inputs) -> np.ndarray:
    from concourse.bass_utils import run_bass_kernel_spmd

    if "nc" not in _CACHE:
        _CACHE["nc"] = _build()
    nc = _CACHE["nc"]

    import os
    in_maps = _prep_inputs(**inputs)
    trace = os.environ.get("BL_TRACE", "0") == "1"
    res = run_bass_kernel_spmd(nc, in_maps, list(range(NCORES)), trace=trace)
    _CACHE["last_result"] = res

    out = np.zeros((4, U, T), dtype=np.float32)
    for q in range(4):
        out[q] = res.results[2 * q]["out"]
    return out
